# revision 14
# baseline (speedup 1.0000x reference)
"""Binary-weight 3x3 conv (depth-1 conv3d), 32ch -> 32ch, on trn2.

Forward pass of a BNN conv: effective weights are scale[o,i] * sign(w[o,i,kh,kw])
(the straight-through-estimator machinery in the reference only affects grads).
Kernel depth is 1, so this is a 2D 3x3 same-padded conv applied independently to
each of N*D = 8*16 = 128 images of shape [32, 160, 160].

Strategy (per core; batch dim sharded 1:1 onto 8 cores), variant "f16":
  - x, w and out travel as fp16 on the wire: the host casts x during the pad
    (input HBM traffic halves), the PE runs fp16 matmuls at 1 cycle/row with
    exact products into the f32 PSUM accumulator, the PSUM->SBUF evacuation
    casts to fp16 (output traffic halves), and the host upcasts the result.
    Total rounding error ~3*2^-11 vs the 2e-2 gate.
  - 16 d-slices per core, processed in 4 groups of 4 images.
  - Images live in SBUF zero-padded to 162 cols, on 32 channels = partitions
    [32r, 32r+32) for image r of the group.
  - PE runs in 32x32 tile-packing mode: tile (r, c) computes image r,
    pixel-segment c. 16 concurrent matmuls per tap, 9 taps accumulate in PSUM
    (tap shifts = free-axis offsets into the padded image); the 4 row-tiles
    of a column group share XBUS beats so a tap streams in ~N cycles.
  - H=160 is covered by 13 12-row rounds plus one exact 4-row remainder
    round (seg_rows=1) -- no row is computed twice.
  - PSUM evacuated to SBUF split between DVE and ACT; per-seg output DMAs
    alternate across the two HWDGE rings while input strip DMAs go through
    the gpsimd SWDGE queues so they never FIFO-block the outputs.
  - Measured on hw: 8064 matmuls + ldweights at ~546ns/tap cadence; the
    tap overhead (sem incs + weight reloads), not stream time or DMA,
    is the binding resource at ~331us.
"""

import numpy as np

import concourse.bass as bass
from concourse.bass import InstructionNameOrderedSet
import concourse.mybir as mybir
import concourse.tile as tile
from concourse import bacc
from concourse import bass_utils

C = 32          # in = out channels
KH = KW = 3


def _matmul_noload(nc, out, lhsT, rhs, tile_position, start, stop):
    """InstMatmult with ldweights=False: uses whatever weights the preceding
    explicit LDWEIGHTS left in this PE tile instead of self-loading. (The
    normal fused path is split into InstLdweights+InstMatmult by a bass
    lowering pass for non-f32 ifmaps; ldweights=False marks the matmul as
    already-non-self-loading.) lhsT stays in `ins` for dep tracking only."""
    eng = nc.tensor
    ifmap_ap = eng.lower_ap(rhs.opt(frozenset({0})), opt=False)
    weights_ap = eng.lower_ap(
        lhsT.opt(frozenset({0})), opt=False, for_matmul_weights=True
    )
    out_ap = eng.lower_ap(out)
    return eng.add_instruction(
        mybir.InstMatmult(
            name=eng.bass.get_next_instruction_name(),
            replication_resolution=0,
            replication_shift_amnt=0,
            replication_num_rows=0,
            start_tensor_calc=start,
            stop_tensor_calc=stop,
            ins=[ifmap_ap, weights_ap],
            outs=[out_ap],
            perf_mode=None,
            is_transpose=None,
            ifmap_quant_offset=None,
            weights_quant_offset=None,
            bass_skip_group_check=True,
            tile_position=tile_position,
            tile_size=(32, 32),
            ldweights=True,
        )
    )

# full-problem dims
FULL_N, FULL_D, FULL_H, FULL_W = 8, 16, 160, 160


def _demote_pe_sync(nc, ins):
    """Demote this instruction's same-engine (PE) sync deps to nosync edges.

    Tile's vector-clock sync makes every instruction with sync descendants
    tick a counting semaphore at completion; on hw each tick costs the PE
    sequencer ~34ns of issue bandwidth, which at 16 matmuls/tap pins the tap
    cadence to ~547ns (the measured baseline bottleneck; stream time is only
    ~200ns). PE->PE edges (PSUM accumulate chains) are enforced for free by
    per-tile in-order execution, so carrying them as nosync (scheduler
    ordering only) drops the dead ticks; cross-engine edges (DMA->mm,
    evac->mm WAR) keep their semaphores. Evac's own sync deps still point at
    the stop matmuls, so those (16/round) keep ticking -- which is exactly
    the set whose completion other engines truly need."""
    sync = ins.take_sync_dependencies()
    keep = InstructionNameOrderedSet()
    demote = InstructionNameOrderedSet()
    for name in sync:
        dep = nc.inst_map.get(name) if hasattr(nc.inst_map, "get") else nc.inst_map[name]
        if dep is not None and dep.engine == mybir.EngineType.PE:
            demote.add(name)
        else:
            keep.add(name)
    ins.set_sync_dependencies(keep)
    if demote:
        ins.add_nosync_dependencies_from(demote)


def _batch_pe_ticks(m):
    """Post-compile IR rewrite: thin out the PE clock-semaphore increments.

    Tile's sync gives EVERY matmul a `PE_sem++ @complete`; on hw the
    sequencer serializes these completion updates at ~34ns each, which at
    144 matmuls/round (16 tiles x 9 taps) pins the kernel to ~547ns/tap
    (the measured baseline bottleneck -- stream time is only ~200ns, and
    LDWEIGHTS, which carry no increment, retire ~free).

    The vector-clock scheme is only sound with in-order per-engine
    retirement (waits are `sem >= k` counting thresholds while tile-packed
    matmuls complete concurrently), so ticks can be thinned: drop the inc
    from every matmul except each round's final one (stop_tensor_calc
    followed by start_tensor_calc = the accumulation-group boundary), then
    renumber every wait threshold on that semaphore into the sparser
    numbering, rounding up to the next kept tick. A waiter thus proceeds
    once the round's last matmul retires, which in-order retirement
    guarantees is after all of the round's matmuls. (The BIR verifier
    requires UpdateValue == 1, so thresholds are renumbered rather than
    batching the count into one fat increment.)"""
    for f in m.functions:
        insts = [i for bb in f.blocks for i in bb.instructions]

        def incs_on(i, sem_id=None):
            si = i.sync_info
            if not si or not si.on_update:
                return []
            return [
                u for u in si.on_update
                if u.sync_type == "semaphore" and u.update_mode == "sem-inc"
                and (sem_id is None or u.id == sem_id)
            ]

        # PE clock semaphores = those incremented by matmuls
        pe_sems = {u.id for i in insts if isinstance(i, mybir.InstMatmult)
                   for u in incs_on(i)}
        for sem in pe_sems:
            carriers = [i for i in insts if incs_on(i, sem)]
            mm_idx = [k for k, i in enumerate(carriers)
                      if isinstance(i, mybir.InstMatmult)]
            if not mm_idx:
                continue
            # keep: all non-matmul carriers; round-final matmuls; the last
            # matmul carrier (so no trailing tick is lost)
            mms = [i for bb in f.blocks for i in bb.instructions
                   if isinstance(i, mybir.InstMatmult)]
            mm_pos = {id(i): k for k, i in enumerate(mms)}
            kept = []
            for k, i in enumerate(carriers):
                if not isinstance(i, mybir.InstMatmult) or k == mm_idx[-1]:
                    kept.append(True)
                    continue
                p = mm_pos[id(i)]
                kept.append(
                    bool(i.stop_tensor_calc) and p + 1 < len(mms)
                    and bool(mms[p + 1].start_tensor_calc)
                )
            # cum_kept[v] = #kept among carriers[0:v]
            cum = [0]
            for k in kept:
                cum.append(cum[-1] + (1 if k else 0))
            n_kept = cum[-1]

            def remap(v):
                # original "first v ticks retired" -> sparser numbering
                if v <= 0:
                    return v
                v = min(v, len(carriers))
                nv = cum[v]
                if not kept[v - 1]:
                    nv += 1          # wait for the next kept tick instead
                return min(nv, n_kept)

            # rewrite waits anywhere in the function
            for i in insts:
                si = i.sync_info
                if not si or not si.on_wait:
                    continue
                if not any(w.sync_type == "semaphore" and w.id == sem
                           for w in si.on_wait):
                    continue
                new_waits = []
                for w in si.on_wait:
                    if w.sync_type == "semaphore" and w.id == sem:
                        assert w.wait_mode == "sem-ge-imm", w.wait_mode
                        w = mybir.SyncWait(
                            sync_type=w.sync_type, id=w.id,
                            ant_name=w.ant_name, wait_mode=w.wait_mode,
                            wait_value=remap(w.wait_value),
                            wait_reg=w.wait_reg,
                        )
                    new_waits.append(w)
                i.sync_info = mybir.SyncInfo(
                    on_wait=new_waits, on_update=list(si.on_update)
                )
            # strip dropped increments
            for k, i in enumerate(carriers):
                if kept[k]:
                    continue
                si = i.sync_info
                drop = set(map(id, incs_on(i, sem)))
                i.sync_info = mybir.SyncInfo(
                    on_wait=list(si.on_wait),
                    on_update=[u for u in si.on_update if id(u) not in drop],
                )


def _round_list(H, seg_rows, SEGS=4):
    """(origin, rows-per-seg) rounds covering H exactly."""
    RPR = SEGS * seg_rows
    rounds = [(j0, seg_rows) for j0 in range(0, H - RPR + 1, RPR)]
    covered = rounds[-1][0] + RPR
    if covered < H:
        assert (H - covered) % SEGS == 0
        rounds.append((covered, (H - covered) // SEGS))
    return rounds


def build_conv(tc, out_ap, x_ap, w_ap, D, H, W, seg_rows, strip_rows, variant):
    """Emit the conv program for one core. x: [32, D, H, W], out: [32, D, H, W].

    variant "f32":   exact fp32 matmuls (4 cyc/row), w: [128, 288] f32.
    variant "bf16x3": x and w each split into bf16 hi+lo; accumulate
        w_hi*x_hi + w_hi*x_lo + w_lo*x_hi (error ~2^-18), w: [128, 2, 288] bf16.
    """
    nc = tc.nc
    f32 = mybir.dt.float32
    bf16 = mybir.dt.bfloat16
    f16 = mybir.dt.float16
    # f16: x/w/out are cast to fp16 on the host, halving HBM traffic in both
    # directions, and the PE runs 1 cycle/row (vs 4 for exact f32). Products
    # are exact in the f32 PSUM accumulator; the only error is the three
    # 2^-11 roundings (x, w, out) -- ~1e-3 vs the 2e-2 gate.
    # (f32r also hits 1 cyc/row but its matmuls fail the s3d3 dst-partition
    # ISA check under 32x32 column tiling, so it cannot be compiled here.)
    x_dt = {
        "f32r": mybir.dt.float32r, "f16": f16, "f16w": f16, "f16r1": f16,
        "f16d": f16, "f16e": f16, "f16g": f16, "f16h": f16, "f16i": f16, "f16j": f16, "f16k": f16, "f16m": f16,
    }.get(variant, f32)
    st_dt = f16 if variant.startswith("f16") else f32

    IPG = 4                      # images per group (row tiles)
    SEGS = 4                     # pixel segments per round (col tiles)
    NMM = seg_rows * W           # moving free size per matmul
    RPR = SEGS * seg_rows        # output rows per round
    assert D % IPG == 0
    NGRP = D // IPG
    WP = W + 2
    assert NMM <= 512

    # rounds as (origin row, rows-per-seg). If RPR doesn't divide H, the
    # remainder is an exact short round (SEGS segs of H%RPR//SEGS rows each)
    # so no row is ever computed twice.
    rounds = _round_list(H, seg_rows, SEGS)
    rounds_per_strip = max(1, strip_rows // RPR)
    strips = [
        rounds[i : i + rounds_per_strip]
        for i in range(0, len(rounds), rounds_per_strip)
    ]
    rspan = lambda rs: rs[-1][0] + SEGS * rs[-1][1] + 2 - rs[0][0]
    XROWS = max(rspan(rs) for rs in strips)

    # per col group, names of the previous tap's matmuls (f16w ordering chain)
    last_mms = [[] for _ in range(SEGS)]
    # x_ap is host-prepadded: [D, C, H+2, W+2] with zero borders, so a strip
    # is one fully-contiguous DMA per partition (partition stride = (H+2)*(W+2)).
    x_r = x_ap.rearrange("(g p) hp wp -> g p (hp wp)", g=NGRP, p=IPG * C)
    # [g] -> (o, r, h*w): row-sliced per round/seg at DMA time. o outermost
    # so the DGE splits each output DMA across all 16 SDMA engines (it
    # splits on the outermost dest dim; with 4 outermost it used only 4)
    if variant in ("f16r1", "f16d", "f16e", "f16g", "f16h", "f16i", "f16j", "f16k", "f16m"):
        out_v = None          # raw [NGRP, NR, 128, IPG, 480] dump layout
    else:
        out_v = out_ap.rearrange("o (g r) h w -> g o r (h w)", g=NGRP, r=IPG)

    xbytes = XROWS * WP * mybir.dt.size(x_dt)
    xbufs = 3 if (variant != "bf16x3" and 3 * xbytes < 160 * 1024) else 2
    # f16e: deeper st/x buffering so the DMA-trigger guards (st-buffer and
    # ring-slot reuse, plus X32 WAR on the PE clock) are satisfied long
    # before they are checked -- a waiting guard head-of-line blocks its
    # whole queue, which was the measured f16d bottleneck.
    stbufs = {"f16e": 6, "f16g": 10, "f16h": 10, "f16i": 5, "f16j": 4, "f16k": 5, "f16m": 4}.get(variant, 3)
    if variant in ("f16e", "f16h", "f16i", "f16j", "f16k", "f16m") and 4 * xbytes < 140 * 1024:
        xbufs = 4
    if variant == "f16g":
        # whole-padded-image group buffers (strip_rows >= H+2): two in
        # flight gives a full group (~24us DMA) of prefetch runway while
        # keeping SBUF at ~105KB/partition for x + ~37KB for st.
        xbufs = 2
    with (
        tc.tile_pool(name="wpool", bufs=1) as wpool,
        tc.tile_pool(name="xpool", bufs=xbufs) as xpool,
        tc.tile_pool(name="stpool", bufs=stbufs) as stpool,
        tc.tile_pool(name="stbpool", bufs=stbufs) as stbpool,
        tc.tile_pool(name="pspool", bufs=2, space="PSUM") as pspool,
    ):
        if variant != "bf16x3":
            w_sb = wpool.tile([128, KH * KW * C], x_dt, tag="w")
        else:
            w_sb = wpool.tile([128, 2, KH * KW * C], bf16, tag="w")
        nc.sync.dma_start(w_sb[:], w_ap[:])

        for g in range(NGRP):
            for si, strip in enumerate(strips):
                X32 = xpool.tile([128, XROWS, WP], x_dt, tag="X32")
                r0 = strip[0][0]
                nrows = rspan(strip)
                # padded rows [r0, r0+nrows) of each image, contiguous runs
                # per partition. The very first strip is split into per-round
                # chunks so round k is gated only on its own rows, not the
                # whole strip. Inputs go through the gpsimd SWDGE queues so
                # the big strip transfers never FIFO-block the output DMAs
                # on the two HWDGE rings.
                if g == 0 and si == 0 and nrows > 3 * RPR:
                    cuts = list(range(RPR + 2, nrows, RPR)) + [nrows]
                    cuts = [0] + [c for c in cuts if c <= nrows]
                elif variant in ("f16i", "f16j", "f16k") and nrows > 2 * RPR:
                    cuts = list(range(2 * RPR + 2, nrows, 2 * RPR)) + [nrows]
                    cuts = [0] + [c for c in cuts if c <= nrows]
                else:
                    cuts = [0, nrows]
                # (measured: routing the first strip's chunks via the sync
                # HWDGE ring instead regresses ~9us -- they collide with the
                # early output DMAs; SWDGE everywhere wins)
                # (measured: first-chunk-on-sync + stpool=4 cost +2us
                # vs this configuration; all-SWDGE inputs + 3 st bufs win)
                in_eng = nc.gpsimd
                for a, b in zip(cuts, cuts[1:]):
                    if b <= a:
                        continue
                    in_eng.dma_start(
                        X32[:, a:b, :].rearrange("p a b -> p (a b)"),
                        x_r[g][:, (r0 + a) * WP : (r0 + b) * WP],
                    )

                if variant != "bf16x3":
                    # comp -> (weight slice index or None, moving buffer)
                    comps = [(None, X32)]
                else:
                    Xhi = xpool.tile([128, XROWS, WP], bf16, tag="Xhi")
                    Xlo = xpool.tile([128, XROWS, WP], bf16, tag="Xlo")
                    nc.scalar.copy(Xhi[:, 0:nrows, :], X32[:, 0:nrows, :])
                    nc.vector.tensor_sub(
                        Xlo[:, 0:nrows, :], X32[:, 0:nrows, :],
                        Xhi[:, 0:nrows, :],
                    )
                    comps = [(0, Xhi), (0, Xlo), (1, Xhi)]

                ri0 = si * rounds_per_strip
                for rk, (j0, sr) in enumerate(strip):
                    ri = ri0 + rk
                    nmm = sr * W
                    ps = pspool.tile([128, SEGS, 512], f32, tag="ps")
                    for ci, (wi, XB) in enumerate(comps):
                        for tap in range(KH * KW):
                            kh, kw = divmod(tap, KW)
                            if variant == "f16w":
                                # one [128,32] LDWEIGHTS per col group fills
                                # all 4 row tiles at once (w_sb partitions are
                                # 4 replicas of the 32-ch weight block); the
                                # 16 matmuls then run without self-loading.
                                # nosync deps pin the per-engine stream order
                                # the IR no longer expresses.
                                lds = []
                                for c in range(SEGS):
                                    ld = nc.tensor.ldweights(
                                        w_sb[:, 32 * tap : 32 * tap + 32],
                                        tile_position=(0, 32 * c),
                                    )
                                    if last_mms[c]:
                                        ld.ins.add_nosync_dependencies_from(
                                            InstructionNameOrderedSet(
                                                last_mms[c]
                                            )
                                        )
                                    lds.append(ld)
                                for c in range(SEGS):
                                    last_mms[c] = []
                                for c in range(SEGS):
                                    for r in range(IPG):
                                        j = j0 - r0 + sr * c
                                        rhs = XB[
                                            32 * r : 32 * r + 32,
                                            j + kh : j + kh + sr,
                                            kw : kw + W,
                                        ]
                                        mm = _matmul_noload(
                                            nc,
                                            ps[32 * c : 32 * c + 32, r, 0:nmm],
                                            w_sb[
                                                32 * r : 32 * r + 32,
                                                32 * tap : 32 * tap + 32,
                                            ],
                                            rhs,
                                            (32 * r, 32 * c),
                                            start=(ci == 0 and tap == 0),
                                            stop=(
                                                ci == len(comps) - 1
                                                and tap == KH * KW - 1
                                            ),
                                        )
                                        mm.ins.add_nosync_dependencies_from(
                                            InstructionNameOrderedSet(
                                                [lds[c].ins.name]
                                            )
                                        )
                                        last_mms[c].append(mm.ins.name)
                                continue
                            # c innermost: consecutive matmuls (and their
                            # legalization-inserted weight loads) hit
                            # different PE COLUMN groups; XBUSes are wired
                            # per column group, so this is what lets the
                            # loads run concurrently instead of serializing
                            # on one column's bus.
                            for r in range(IPG):
                                for c in range(SEGS):
                                    if wi is None:
                                        lhsT = w_sb[
                                            32 * r : 32 * r + 32,
                                            32 * tap : 32 * tap + 32,
                                        ]
                                    else:
                                        lhsT = w_sb[
                                            32 * r : 32 * r + 32, wi,
                                            32 * tap : 32 * tap + 32,
                                        ]
                                    j = j0 - r0 + sr * c
                                    rhs = XB[
                                        32 * r : 32 * r + 32,
                                        j + kh : j + kh + sr,
                                        kw : kw + W,
                                    ]
                                    mm = nc.tensor.matmul(
                                        ps[32 * c : 32 * c + 32, r, 0:nmm],
                                        lhsT,
                                        rhs,
                                        start=(ci == 0 and tap == 0),
                                        stop=(
                                            ci == len(comps) - 1
                                            and tap == KH * KW - 1
                                        ),
                                        tile_position=(32 * r, 32 * c),
                                    )
                                    if variant not in ("f32", "f32r", "f16", "f16w", "f16r1", "bf16x3"):
                                        _demote_pe_sync(nc, mm.ins)
                    if variant == "f16m":
                        # f16j's 3-round batched outputs (5760B bursts,
                        # ~267GB/s measured) with both chunk DMAs on the SP
                        # queue so the ACT queue stays clean, plus
                        # alternating evac emission order per round to break
                        # the scheduler's CAST->ACTIVATE psum-reader chain.
                        ck = ri % 3
                        if ck == 0:
                            stA = stpool.tile([128, 3, 2, NMM], st_dt, tag="stA")
                            stB = stbpool.tile([128, 3, 2, NMM], st_dt, tag="stB")
                            ck0 = ri
                        evacs = [
                            (nc.vector.tensor_copy, stA, 0),
                            (nc.scalar.copy, stB, 2),
                        ]
                        if ri % 2:
                            evacs.reverse()
                        for fn, stx, p0 in evacs:
                            fn(stx[:, ck, :, 0:nmm], ps[:, p0 : p0 + 2, 0:nmm])
                        NR_G = len(rounds)
                        if ck == 2 or ri == NR_G - 1:
                            n_ck = ri - ck0 + 1
                            nc.sync.dma_start(
                                out_ap[g, 0, :, ck0 : ri + 1, :, :],
                                stA[:, 0:n_ck, :, :],
                            )
                            nc.sync.dma_start(
                                out_ap[g, 1, :, ck0 : ri + 1, :, :],
                                stB[:, 0:n_ck, :, :],
                            )
                        continue
                    if variant == "f16j":
                        # 3-round output chunks: each evac half owns a
                        # [128, 3, 2, nmm] super-tile; one DMA per chunk per
                        # half writes 5760B contiguous per partition (vs
                        # 1920B per-round), and the halves ride different
                        # HWDGE rings (SP / ACT) in parallel.
                        ck = ri % 3
                        if ck == 0:
                            stA = stpool.tile([128, 3, 2, NMM], st_dt, tag="stA")
                            stB = stbpool.tile([128, 3, 2, NMM], st_dt, tag="stB")
                            ck0 = ri
                        nc.vector.tensor_copy(
                            stA[:, ck, :, 0:nmm], ps[:, 0:2, 0:nmm]
                        )
                        nc.scalar.copy(
                            stB[:, ck, :, 0:nmm], ps[:, 2:4, 0:nmm]
                        )
                        NR_G = len(rounds)
                        if ck == 2 or ri == NR_G - 1:
                            n_ck = ri - ck0 + 1
                            nc.sync.dma_start(
                                out_ap[g, 0, :, ck0 : ri + 1, :, :],
                                stA[:, 0:n_ck, :, :],
                            )
                            nc.scalar.dma_start(
                                out_ap[g, 1, :, ck0 : ri + 1, :, :],
                                stB[:, 0:n_ck, :, :],
                            )
                        continue
                    if variant == "f16k":
                        # f16i structure, but the B half rides the ACT
                        # HWDGE ring: two rings in parallel double the
                        # output path's burst-limited (~171GB/s @1920B)
                        # per-ring rate.
                        stA = stpool.tile([128, 2, nmm], st_dt, tag="stA")
                        stB = stbpool.tile([128, 2, nmm], st_dt, tag="stB")
                        nc.vector.tensor_copy(stA[:, :, :], ps[:, 0:2, 0:nmm])
                        nc.sync.dma_start(
                            out_ap[g, ri, :, 0:2, 0:nmm], stA[:, :, :],
                        )
                        nc.scalar.copy(stB[:, :, :], ps[:, 2:4, 0:nmm])
                        nc.scalar.dma_start(
                            out_ap[g, ri, :, 2:4, 0:nmm], stB[:, :, :],
                        )
                        continue
                    if variant == "f16i":
                        # decoupled evac halves: each engine owns its tile +
                        # output DMA, so no shared-tile edge can serialize
                        # ACT behind DVE (the measured f16e pattern)
                        stA = stpool.tile([128, 2, nmm], st_dt, tag="stA")
                        stB = stbpool.tile([128, 2, nmm], st_dt, tag="stB")
                        nc.vector.tensor_copy(stA[:, :, :], ps[:, 0:2, 0:nmm])
                        nc.sync.dma_start(
                            out_ap[g, ri, :, 0:2, 0:nmm], stA[:, :, :],
                        )
                        nc.scalar.copy(stB[:, :, :], ps[:, 2:4, 0:nmm])
                        nc.sync.dma_start(
                            out_ap[g, ri, :, 2:4, 0:nmm], stB[:, :, :],
                        )
                        continue
                    st = stpool.tile([128, SEGS, nmm], st_dt, tag="st")
                    nc.vector.tensor_copy(st[:, 0:2, :], ps[:, 0:2, 0:nmm])
                    nc.scalar.copy(st[:, 2:4, :], ps[:, 2:4, 0:nmm])
                    if variant in ("f16e", "f16g", "f16h"):
                        # two DMAs per round into the raw dump layout, each
                        # launched as soon as its own evac half lands, both
                        # triggered from the otherwise-idle SP queue so the
                        # DVE/ACT evac queues never sit behind DMA guards.
                        nc.sync.dma_start(
                            out_ap[g, ri, :, 0:2, 0:nmm], st[:, 0:2, :],
                        )
                        nc.sync.dma_start(
                            out_ap[g, ri, :, 2:4, 0:nmm], st[:, 2:4, :],
                        )
                        continue
                    if variant in ("f16r1", "f16d"):
                        # ONE dma per round into the raw dump layout
                        # [g, ri, 128, r, q]; the host unscrambles. Quarters
                        # HWDGE ring occupancy vs 4 per-seg DMAs.
                        eng = nc.sync if ri % 2 == 0 else nc.scalar
                        eng.dma_start(
                            out_ap[g, ri, :, :, 0:nmm],
                            st[:, :, :],
                        )
                        continue
                    for c in range(SEGS):
                        eng = nc.sync if c % 2 == 0 else nc.scalar
                        lo_px = (j0 + sr * c) * W
                        eng.dma_start(
                            out_v[g][:, :, lo_px : lo_px + nmm],
                            st[32 * c : 32 * c + 32, :, :],
                        )


def build_module(n_cores=8, D=FULL_D, H=FULL_H, W=FULL_W, seg_rows=3,
                 strip_rows=None, variant="f32"):
    if strip_rows is None:
        strip_rows = {"bf16x3": 36, "f16g": 168}.get(variant, 96)
    nc = bacc.Bacc(
        "TRN2",
        target_bir_lowering=False,
        debug=False,
        num_devices=n_cores,
    )
    in_dt = {
        "f32r": mybir.dt.float32r, "f16": mybir.dt.float16,
        "f16w": mybir.dt.float16, "f16r1": mybir.dt.float16,
        "f16d": mybir.dt.float16, "f16e": mybir.dt.float16,
        "f16g": mybir.dt.float16, "f16h": mybir.dt.float16,
        "f16i": mybir.dt.float16, "f16j": mybir.dt.float16,
        "f16k": mybir.dt.float16, "f16m": mybir.dt.float16,
    }.get(variant, mybir.dt.float32)
    out_dt = (
        mybir.dt.float16 if variant.startswith("f16") else mybir.dt.float32
    )
    x_d = nc.dram_tensor(
        "x", [D * C, H + 2, W + 2], in_dt, kind="ExternalInput"
    )
    if variant != "bf16x3":
        w_d = nc.dram_tensor(
            "w", [128, KH * KW * C], in_dt, kind="ExternalInput"
        )
    else:
        w_d = nc.dram_tensor(
            "w", [128, 2, KH * KW * C], mybir.dt.bfloat16, kind="ExternalInput"
        )
    if variant in ("f16j", "f16m"):
        nr = len(_round_list(H, seg_rows))
        out_d = nc.dram_tensor(
            "out", [D // 4, 2, 128, nr, 2, 3 * W], out_dt,
            kind="ExternalOutput"
        )
    elif variant in ("f16r1", "f16d", "f16e", "f16g", "f16h", "f16i", "f16k"):
        nr = len(_round_list(H, seg_rows))
        out_d = nc.dram_tensor(
            "out", [D // 4, nr, 128, 4, 3 * W], out_dt, kind="ExternalOutput"
        )
    else:
        out_d = nc.dram_tensor(
            "out", [C, D, H, W], out_dt, kind="ExternalOutput"
        )
    with tile.TileContext(nc) as tc:
        build_conv(
            tc, out_d.ap(), x_d.ap(), w_d.ap(), D, H, W, seg_rows, strip_rows,
            variant,
        )
    nc.compile()
    if variant not in ("f32", "f32r", "f16", "f16w", "f16r1", "bf16x3"):
        _batch_pe_ticks(nc.m)
    return nc


def binarize_weights(weights, variant="bf16x3"):
    """Host-side: [32,32,1,3,3] fp32 -> packed replicated weight tile.
    w_packed[32r+i, 32*tap+o] = scale[o,i] * sign(w[o,i,kh,kw]), tap = kh*3+kw.
    f32: [128, 288] f32.  bf16x3: [128, 2, 288] bf16 (hi, lo split)."""
    w = np.asarray(weights, dtype=np.float32)
    scale = np.mean(np.abs(w), axis=(2, 3, 4), keepdims=True)
    bw = (scale * np.sign(w)).astype(np.float32)          # [o, i, 1, 3, 3]
    wt = bw[:, :, 0].transpose(1, 2, 3, 0).reshape(C, KH * KW * C)  # [i, tap*32+o]
    full = np.ascontiguousarray(np.tile(wt, (4, 1)))       # [128, 288] f32
    if variant in ("f32", "f32r"):
        return full
    if variant.startswith("f16"):
        return full.astype(np.float16)
    import ml_dtypes
    hi = full.astype(ml_dtypes.bfloat16)
    lo = (full - hi.astype(np.float32)).astype(ml_dtypes.bfloat16)
    return np.ascontiguousarray(np.stack([hi, lo], axis=1))  # [128, 2, 288] bf16


_NC_CACHE = {}


def _get_nc(key, **kwargs):
    if key not in _NC_CACHE:
        _NC_CACHE[key] = build_module(**kwargs)
    return _NC_CACHE[key]


def pad_input(x, np_dt=np.float32):
    """[N, C, D, H, W] f32 -> [N, D*C, H+2, W+2] zero-padded, d-major."""
    n, c, d, h, w = x.shape
    xp = np.zeros((n, d, c, h + 2, w + 2), dtype=np_dt)
    xp[:, :, :, 1 : h + 1, 1 : w + 1] = x.transpose(0, 2, 1, 3, 4)
    return xp.reshape(n, d * c, h + 2, w + 2)


def run(x, weights, trace=False, variant="f16r1", seg_rows=3, strip_rows=None):
    x = np.asarray(x, dtype=np.float32)
    n_cores = x.shape[0]
    key = (n_cores, variant, seg_rows, strip_rows)
    nc = _get_nc(
        key, n_cores=n_cores, seg_rows=seg_rows, strip_rows=strip_rows,
        variant=variant,
    )
    xp = pad_input(
        x, np.float16 if variant.startswith("f16") else np.float32
    )
    w_packed = binarize_weights(weights, variant)
    in_maps = [{"x": xp[n], "w": w_packed} for n in range(n_cores)]
    res = bass_utils.run_bass_kernel_spmd(
        nc, in_maps, core_ids=list(range(n_cores)), trace=trace
    )
    out = np.stack([res.results[n]["out"] for n in range(n_cores)])
    if variant in ("f16j", "f16m"):
        out = _unscramble_j(out)
    elif variant in ("f16r1", "f16d", "f16e", "f16g", "f16h", "f16i", "f16k"):
        out = _unscramble(out)
    if out.dtype != np.float32:
        out = out.astype(np.float32)
    return out, res


def _unscramble(raw, H=FULL_H, W=FULL_W, seg_rows=3):
    """[n, NGRP, NR, 128, 4, 3W] f16 raw dump -> [n, C, D, H, W] f32.
    raw[n, g, ri, 32c+o, r, u*W+v] = out[n, o, 4g+r, j0(ri)+sr*c+u, v]."""
    n, ngrp, nr = raw.shape[:3]
    rounds = _round_list(H, seg_rows)
    out = np.empty((n, C, ngrp * 4, H, W), dtype=np.float32)
    full = [k for k, (_, sr) in enumerate(rounds) if sr == seg_rows]
    assert full == list(range(len(full)))
    nf = len(full)
    f = raw[:, :, :nf].reshape(n, ngrp, nf, 4, C, 4, seg_rows, W)
    # [n, g, ri, c, o, r, u, v] -> [n, o, g, r, ri, c, u, v]
    out[:, :, :, : nf * 4 * seg_rows, :] = (
        f.transpose(0, 4, 1, 5, 2, 3, 6, 7)
        .reshape(n, C, ngrp * 4, nf * 4 * seg_rows, W)
    )
    for k in range(nf, nr):
        j0, sr = rounds[k]
        s = raw[:, :, k, :, :, : sr * W].reshape(
            n, ngrp, 4, C, 4, sr, W
        )
        out[:, :, :, j0 : j0 + 4 * sr, :] = (
            s.transpose(0, 3, 1, 4, 2, 5, 6)
            .reshape(n, C, ngrp * 4, 4 * sr, W)
        )
    return out


def _unscramble_j(raw, H=FULL_H, W=FULL_W, seg_rows=3):
    """[n, NGRP, 2, 128, NR, 2, 3W] f16 raw dump -> [n, C, D, H, W] f32.
    raw[n, g, h, 32c+o, ri, r2, u*W+v] = out[n, o, 4g+2h+r2, j0(ri)+sr*c+u, v]
    (u < sr; remainder rounds use only the first sr*W of the 3W slot)."""
    n, ngrp = raw.shape[:2]
    rounds = _round_list(H, seg_rows)
    nr = len(rounds)
    out = np.empty((n, C, ngrp * 4, H, W), dtype=np.float32)
    full = [k for k, (_, sr) in enumerate(rounds) if sr == seg_rows]
    assert full == list(range(len(full)))
    nf = len(full)
    f = raw[:, :, :, :, :nf].reshape(n, ngrp, 2, 4, C, nf, 2, seg_rows, W)
    # [n,g,h,c,o,ri,r2,u,v] -> [n,o,g,h,r2,ri,c,u,v]
    out[:, :, :, : nf * 4 * seg_rows, :] = (
        f.transpose(0, 4, 1, 2, 6, 5, 3, 7, 8)
        .reshape(n, C, ngrp * 4, nf * 4 * seg_rows, W)
    )
    for k in range(nf, nr):
        j0, sr = rounds[k]
        s = raw[:, :, :, :, k, :, : sr * W].reshape(
            n, ngrp, 2, 4, C, 2, sr, W
        )
        out[:, :, :, j0 : j0 + 4 * sr, :] = (
            s.transpose(0, 4, 1, 2, 5, 3, 6, 7)
            .reshape(n, C, ngrp * 4, 4 * sr, W)
        )
    return out


def kernel(x, weights):
    out, _ = run(x, weights)
    return out

